# revision 10
# baseline (speedup 1.0000x reference)
"""One-sided Chamfer distance on 8 Trainium2 NeuronCores.

Math: for each point x in set1, d2(x) = min_j ||x - q_j||^2
            = sq1(x) + min_j (sq2(j) - 2 x.q_j)
so the device only needs  min_j e[i,j]  with  e[i,j] = sq2(j) - 2 x_i.q_j,
which is a rank-K matmul (K = a few augmented rows) followed by a min-reduce.
The cheap tail (add sq1, clamp, sqrt, global sum over 16K values) runs on host.

Precision: the PE's fp32 matmul path costs 4 cyc/row, so inputs are split
into bf16 (hi, lo) pairs and the product is computed as
(xh+xl).(Qh+Ql) + Sh + Sl via a K=14 bf16 matmul accumulated in fp32 PSUM
(~17-18 bit effective input precision, final rel err ~1e-5).

Sharding: core c handles batch c//4, set1 rows [ (c%4)*2048 : (c%4+1)*2048 ),
with its batch's full set2 replicated (sharding hint).

Device dataflow per core (SPMD, same program):
  - s2m  [14, 8192] bf16: set2-side moving operand (rows: Qh,Ql,Qh,Ql,Sh,Sl)
  - w    [14, 2048] bf16: set1-side stationary operand (rows: xh,xh,xl,xl,1,1)
  - 16 i-tiles x 16 matmuls [K=14,M=128]x[K=14,N=512] -> PSUM fp32
  - ACT evicts every second [128,1024] PSUM tile to SBUF (PSUM has a single
    DVE read port, so a 2-PSUM-input DVE op is illegal).
  - DVE tensor_tensor_reduce(op0=min, op1=min) consumes (psum, sbuf) tile
    PAIRS -> 2 fresh elements/lane/cycle, with the accumulator chained
    through the per-partition `scalar` init.
  - out  [128, 16] fp32: column t = min_j e for point i = t*128 + partition
"""

import numpy as np
import ml_dtypes

import concourse.bass as bass
import concourse.mybir as mybir
import concourse.tile as tile
from concourse import bacc
from concourse import dve_ops as _dops
from concourse.bass_utils import run_bass_kernel_spmd
from concourse.dve_spec import C0 as _C0, Spec as _Spec, Src0 as _Src0, \
    Src1 as _Src1, lower as _dve_lower, minn as _minn
from concourse.dve_table_gen import dve_ver_for as _dve_ver_for
from concourse.dve_uop import DveOpSpec as _DveOpSpec

NCORES = 8
B = 2          # batches
N = 8192       # set1 points per batch
M = 8192       # set2 points per batch
D = 3
SLICES = NCORES // B          # 4 set1 slices per batch
NI = N // SLICES              # 2048 set1 points per core
K = 14                        # augmented contraction dim
P = 128
NT = NI // P                  # 16 i-tiles per core
MM_N = 512                    # matmul free width (one PSUM bank)
JC = 1024                     # TTR operand width (2 PSUM banks)
OPS_PER_TILE = M // (2 * JC)  # 4 chained reduce ops per i-tile

_bf16 = ml_dtypes.bfloat16
FLT_BIG = 3.0e38

_CACHED_NC = None


def _ref_tt_min_min(in0, in1, s0, s1, imm2):
    """CoreSim reference: body = min(in0, in1); accum = min(s0, min_k body)."""
    body = np.minimum(in0.astype(np.float32), np.asarray(in1, np.float32))
    body = body.astype(np.float32)
    red = body.reshape(body.shape[0], -1).min(axis=-1, keepdims=True)
    return body, np.minimum(np.asarray(s0, np.float32), red).astype(np.float32)


def _register_min_op():
    """Register a custom DVE op: out = min(in0, in1); accum_out = min(s0,
    min_k out). Ingests one PSUM + one SBUF stream per cycle and fuses the
    full min-reduce — the native TENSOR_TENSOR_REDUCE ISA op faults at
    runtime on this deployment, so we ship our own uop program instead."""
    for o in _dops.OPS:
        if o.name == "TT_MIN_MIN_ANT":
            return o
    op = _dops.DveOp(
        "TT_MIN_MIN_ANT",
        _Spec(
            body=_minn(_Src0, _Src1),
            accum=_minn,
            accum_init=_C0,
            reference=_ref_tt_min_min,
        ),
        subdim=False,
        uops_sha={},
    )
    _dops.OPS.append(op)
    _dops.CUSTOM_DVE_SPECS[op.name] = op.spec
    _dops._SUB_OPCODE_FOR_NAME[op.name] = (
        _dops._CUSTOM_DVE_ROW_BASE + len(_dops.OPS) - 1
    )
    for trn in ("TRN2",):
        ver = _dve_ver_for(trn)
        tmp = _DveOpSpec(
            name=op.name,
            opcode=_dops.get_dve_sub_opcode(op.name),
            uops=_dve_lower(op.spec, ver=ver),
            rd1_en=True,
        )
        op.uops_sha[ver] = tmp.sha(ver)
    return op


def _build_bass():
    min_op = _register_min_op()
    nc = bacc.Bacc("TRN2", target_bir_lowering=False, debug=False,
                   num_devices=NCORES)
    s2m_d = nc.dram_tensor("s2m", [K, M], mybir.dt.bfloat16,
                           kind="ExternalInput")
    w_d = nc.dram_tensor("w", [K, NI], mybir.dt.bfloat16,
                         kind="ExternalInput")
    out_d = nc.dram_tensor("out", [P, NT], mybir.dt.float32,
                           kind="ExternalOutput")

    with tile.TileContext(nc) as tc:
        with tc.tile_pool(name="const", bufs=1) as cpool, \
             tc.tile_pool(name="acc", bufs=4) as apool, \
             tc.tile_pool(name="evict", bufs=3) as epool, \
             tc.tile_pool(name="psum", bufs=2, space="PSUM") as ppool:
            s2m = cpool.tile([K, M], mybir.dt.bfloat16)
            nc.sync.dma_start(out=s2m, in_=s2m_d.ap())
            w = cpool.tile([K, NI], mybir.dt.bfloat16)
            nc.sync.dma_start(out=w, in_=w_d.ap())
            mins = cpool.tile([P, NT], mybir.dt.float32)

            for t in range(NT):
                wt = w[:, t * P:(t + 1) * P]
                prev = None
                for o in range(OPS_PER_TILE):
                    pa = ppool.tile([P, JC], mybir.dt.float32, tag="pa")
                    pb = ppool.tile([P, JC], mybir.dt.float32, tag="pb")
                    j0 = o * 2 * JC
                    for half, pt in ((0, pa), (1, pb)):
                        base = j0 + half * JC
                        nc.tensor.matmul(pt[:, 0:MM_N], wt,
                                         s2m[:, base:base + MM_N],
                                         start=True, stop=True)
                        nc.tensor.matmul(pt[:, MM_N:JC], wt,
                                         s2m[:, base + MM_N:base + JC],
                                         start=True, stop=True)
                    # ACT evicts pb to SBUF (PSUM has one DVE read port).
                    sb = epool.tile([P, JC], mybir.dt.float32, tag="sb")
                    nc.scalar.copy(sb, pb)
                    scratch = epool.tile([P, JC], mybir.dt.float32, tag="scr")
                    if o == OPS_PER_TILE - 1:
                        accum = mins[:, t:t + 1]
                    else:
                        accum = apool.tile([P, 1], mybir.dt.float32, tag="acc")
                    nc.vector._custom_dve(
                        min_op,
                        out=scratch,
                        in0=pa,
                        in1=sb,
                        s0=(FLT_BIG if prev is None else prev),
                        accum_out=accum,
                    )
                    prev = accum

            nc.sync.dma_start(out=out_d.ap(), in_=mins)
    nc.compile()
    return nc


def _split_bf16(x64):
    """x (float64) -> (hi, lo) bf16 pair; hi+lo approximates x to ~16-17 bits."""
    hi = x64.astype(np.float32).astype(_bf16)
    lo = (x64 - hi.astype(np.float64)).astype(np.float32).astype(_bf16)
    return hi, lo


def _prep_inputs(set1, set2):
    """Build per-core {s2m, w} maps (bf16) for the 8 SPMD cores."""
    set1 = np.asarray(set1, np.float32)
    set2 = np.asarray(set2, np.float32)
    s2m_per_batch = []
    for b in range(B):
        q64 = set2[b].astype(np.float64)           # [M, 3]
        Qh, Ql = _split_bf16(-2.0 * q64)           # [M, 3] each
        Sh, Sl = _split_bf16((q64 * q64).sum(-1))  # [M] each
        s2m = np.empty((K, M), dtype=_bf16)
        s2m[0:3] = Qh.T
        s2m[3:6] = Ql.T
        s2m[6:9] = Qh.T
        s2m[9:12] = Ql.T
        s2m[12] = Sh
        s2m[13] = Sl
        s2m_per_batch.append(np.ascontiguousarray(s2m))

    in_maps = []
    for c in range(NCORES):
        b, s = divmod(c, SLICES)
        x64 = set1[b, s * NI:(s + 1) * NI].astype(np.float64)  # [NI, 3]
        xh, xl = _split_bf16(x64)
        w = np.empty((K, NI), dtype=_bf16)
        w[0:3] = xh.T
        w[3:6] = xh.T
        w[6:9] = xl.T
        w[9:12] = xl.T
        w[12] = _bf16(1.0)
        w[13] = _bf16(1.0)
        in_maps.append({"s2m": s2m_per_batch[b], "w": np.ascontiguousarray(w)})
    return in_maps


def _postprocess(set1, results):
    """Host tail: d = sqrt(max(sq1 + min_e, 0)); return fp32 sum."""
    set1 = np.asarray(set1, np.float32)
    total = 0.0
    for c in range(NCORES):
        b, s = divmod(c, SLICES)
        mins = np.asarray(results[c]["out"])          # [P, NT]
        mins_flat = mins.T.reshape(-1)                # index t*128+p -> i
        x = set1[b, s * NI:(s + 1) * NI]
        sq1 = (x.astype(np.float64) ** 2).sum(-1)
        d2 = np.maximum(sq1 + mins_flat.astype(np.float64), 0.0)
        total += np.sqrt(d2).sum()
    return np.asarray(total, dtype=np.float32)


def kernel(set1, set2):
    global _CACHED_NC
    if _CACHED_NC is None:
        _CACHED_NC = _build_bass()
    in_maps = _prep_inputs(set1, set2)
    res = run_bass_kernel_spmd(_CACHED_NC, in_maps, core_ids=list(range(NCORES)))
    return _postprocess(set1, res.results)
